# revision 2
# baseline (speedup 1.0000x reference)
"""AVWGCN Trainium2 kernel, v3: SBUF-resident graph conv, fp8 phase-3.

Math (per batch b, node n):
    S = E @ E.T                        [N, N]  (symmetric)
    Mexp = exp(relu(S)) = max(exp(S),1)
    R = rowsum(Mexp);  A = diag(1/R) Mexp     (softmax(relu(S)))
    z0 = x ; z1 = A x ; z2 = 2 A z1 - x
    out[b,n,o] = sum_d E[n,d] * (sum_ki z_k[b,n,i] Wp[d,k,i,o]) + (E bp)[n,o]

Design vs baseline:
  * Everything stays in SBUF (no Mexp DRAM spill).
  * Phase 3 (z1 = A x): A^T stored column-normalized fp8e4m3
    (AT[m,n] = 128*A[n,m], diagonal zeroed), DoubleRow fp8 matmuls over
    (x_fp8 + x_residual_fp8); exact diagonal a*x added on DVE.
  * Phase 4 (A z1) in bf16 against the raw symmetric Mexp tiles, with
    1/R applied per-partition in the stitch (z2 = 2*r*psum - x).
  * exp(relu(s)) computed as max(exp(s),1); rowsums via 1-partition
    ones-matmuls on PE (columns sums = row sums by symmetry).
  * All transposes via DMA xbar; host pre-transposes x / casts dtypes;
    output written [N, B_local, CO], untransposed on host.
"""
import sys

if "/opt/trn_rl_repo" not in sys.path:
    sys.path.insert(0, "/opt/trn_rl_repo")

import numpy as np
import ml_dtypes
import bass_rust
import concourse.bass as bass
import concourse.mybir as mybir
from concourse import tile
from concourse.vector_clock import ScopedClock
from concourse.bass_utils import run_bass_kernel_spmd

F32 = mybir.dt.float32
F32R = mybir.dt.float32r
BF16 = mybir.dt.bfloat16
F8E4 = mybir.dt.float8e4
ALU = mybir.AluOpType
AF = mybir.ActivationFunctionType
DR = mybir.MatmulPerfMode.DoubleRow

B, N, CI, CO, KCH, D = 64, 2048, 64, 64, 3, 10
NCORES = 8
BL = B // NCORES          # 8 local batches
BC = BL * CI              # 512
NT = N // 128             # 16 node chunks
DO = D * CO               # 640
KA = 128.0                # fp8 A scale

# engine splits (tunables)
EVAC_ACT = 8              # ACT evacs per 8 (all ACT)
DCON_DVE = (0, 1, 2, 3, 4)     # d's via DVE fused stt
DCON_POOL = (5, 6, 7, 8, 9)    # d's via Pool scale + DVE fold


# ---------------------------------------------------------------- env patches
def _patched_drain_and_barrier(self, tick_clock, wait_clock):
    nop_inst = self.nc.sync.nop(nofuse=True, hint="tile_tail_wait")
    wait_clock.add_sem_waits(nop_inst.ins,
                             ScopedClock({None: tick_clock.global_clock}))
    si = nop_inst.ins.sync_info
    waits = list(si.on_wait) if si is not None else []
    if len(waits) > 1:
        nop_inst.ins.sync_info = bass_rust.SyncInfo(
            on_wait=waits[:1], on_update=list(si.on_update))
        for w in waits[1:]:
            extra = self.nc.sync.nop(nofuse=True, hint="tile_tail_wait_x")
            extra.ins.sync_info = bass_rust.SyncInfo(on_wait=[w], on_update=[])
    self.nc.sync.drain()
    self.nc.all_engine_barrier()
    assert self.sems is not None
    popped = self.nc._tile_sem_poison_stack.pop()
    assert popped is self._sem_poison
    self.nc.clear_and_free_semaphores(list(self.sems.allocated().values()))
    self.nc.all_engine_barrier()


tile.TileContext._drain_and_barrier = _patched_drain_and_barrier


def split_multi_waits(nc):
    for f in nc.m.functions:
        for bb in f.blocks:
            new = []
            for inst in bb.instructions:
                si = inst.sync_info
                if si is not None and len(si.on_wait) > 1:
                    waits = list(si.on_wait)
                    for w in waits[:-1]:
                        nop = mybir.InstNoOp(
                            name=nc.get_next_instruction_name(), ins=[], outs=[])
                        nop.engine = inst.engine
                        nop.sync_info = bass_rust.SyncInfo(on_wait=[w], on_update=[])
                        new.append(nop)
                    inst.sync_info = bass_rust.SyncInfo(
                        on_wait=[waits[-1]], on_update=list(si.on_update))
                new.append(inst)
            bb.instructions = new


# ---------------------------------------------------------------- kernel body
def build_nc():
    nc = bass.Bass()

    xbf_d = nc.dram_tensor("xbf", [N, BC], BF16, kind="ExternalInput")
    xf8_d = nc.dram_tensor("xf8", [N, BC], F8E4, kind="ExternalInput")
    xr8_d = nc.dram_tensor("xr8", [N, BC], F8E4, kind="ExternalInput")
    embT_d = nc.dram_tensor("embT", [D, N], F32, kind="ExternalInput")
    emb_d = nc.dram_tensor("emb", [N, D], F32, kind="ExternalInput")
    wp_d = nc.dram_tensor("wp", [D, KCH, CI, CO], F32, kind="ExternalInput")
    bp_d = nc.dram_tensor("bp", [D, CO], BF16, kind="ExternalInput")
    mask_d = nc.dram_tensor("maskd", [128, 128], BF16, kind="ExternalInput")
    out_d = nc.dram_tensor("out_l", [N, BL, CO], F32, kind="ExternalOutput")

    with tile.TileContext(nc) as tc:
        with (
            tc.tile_pool(name="const", bufs=1) as cpool,
            tc.tile_pool(name="big", bufs=1) as bpool,
        ):
            # ---- constants -------------------------------------------------
            et = cpool.tile([16, N], F32R)             # E^T rows 0:10 (f32r)
            etb = cpool.tile([16, N], BF16)            # E^T bf16 (bias mm)
            e_sb = cpool.tile([128, NT * D], F32)      # E as [p, nt*10+d]
            bias_sb = cpool.tile([128, NT * CO], F32)
            bp_sb = cpool.tile([16, CO], BF16)
            wr12 = cpool.tile([128, DO], BF16)         # [W1;W2] stacked (ki)
            wr0 = cpool.tile([128, DO], BF16)          # W0 duplicated
            rrep = cpool.tile([128, N], BF16)          # 128/R[n], bcast rows
            rd_s = cpool.tile([128, N], BF16)          # xbar of rrep
            ones = cpool.tile([128, 128], BF16)        # 1/128 each
            maskI = cpool.tile([128, 128], BF16)       # 1 - I
            amn = cpool.tile([128, NT], F32)           # a[n] = Mexp_nn / R
            rd2 = cpool.tile([128, NT], F32)           # 2/R[n] ([p,t] layout)
            diagm = cpool.tile([128, NT], BF16)
            e2 = cpool.tile([128, NT * D], F32)
            nrm = cpool.tile([128, NT], F32)

            # ---- big persistent tiles -------------------------------------
            atb = bpool.tile([128, NT * N], F8E4, name="atb")   # A^T * 128
            xbf = bpool.tile([128, NT * BC], BF16, name="xbf")
            xf8 = bpool.tile([128, NT * BC], F8E4, name="xf8")
            xr8 = bpool.tile([128, NT * BC], F8E4, name="xr8")
            z1bf = bpool.tile([128, NT * BC], BF16, name="z1bf")
            z0t = bpool.tile([128, 4 * N], BF16, name="z0t")

            atv = atb[:].rearrange("p (mt n) -> p mt n", mt=NT)
            xfv = xf8[:].rearrange("p (mt c) -> p mt c", mt=NT)
            xrv = xr8[:].rearrange("p (mt c) -> p mt c", mt=NT)
            z0v = z0t[:].rearrange("p (w n) -> p w n", w=4)
            # z12t reuses atb's bytes after phase 3 is done with it
            z12t = atb[:].bitcast(BF16)
            z12v = z12t.rearrange("p (b n) -> p b n", b=BL)
            rdv = rd_s[:].rearrange("p (w j) -> p w j", w=NT)

            nc.sync.dma_start(et[0:D, :], embT_d[:].bitcast(F32R))
            nc.vector.tensor_copy(etb[0:D, :], et[0:D, :].bitcast(F32))
            nc.sync.dma_start(
                e_sb[:].rearrange("p (nt d) -> p nt d", nt=NT),
                emb_d[:].rearrange("(nt p) d -> p nt d", p=128))
            nc.sync.dma_start(bp_sb[0:D, :], bp_d[:])
            nc.sync.dma_start(maskI[:], mask_d[:])

            # x loads (SWDGE queue; SP stays free for phase-1 DMAs)
            for t, src in ((xbf, xbf_d), (xf8, xf8_d), (xr8, xr8_d)):
                nc.gpsimd.dma_start(
                    t[:].rearrange("p (mt c) -> p mt c", mt=NT),
                    src[:].rearrange("(mt p) c -> p mt c", p=128))

            # weights: load W0,W1,W2 as [i, (d,o)] then cast to bf16
            with tc.tile_pool(name="wload", bufs=1) as wpool:
                wtmp = []
                for k in range(KCH):
                    wf = wpool.tile([64, DO], F32, tag=f"wload{k}")
                    nc.sync.dma_start(
                        wf[:].rearrange("p (d o) -> p d o", d=D),
                        wp_d[:, k, :, :].transpose([1, 0, 2]))
                    wtmp.append(wf)
                nc.vector.tensor_copy(wr12[0:64, :], wtmp[1][:])
                nc.vector.tensor_copy(wr12[64:128, :], wtmp[2][:])
                nc.vector.tensor_copy(wr0[0:64, :], wtmp[0][:])
                nc.vector.tensor_copy(wr0[64:128, :], wtmp[0][:])

            # xT transposes into z0t (DMA xbar), as soon as xbf lands
            for nt in range(NT):
                nc.sync.dma_start_transpose(
                    z0v[:, :, nt * 128:(nt + 1) * 128],
                    xbf[:, nt * BC:(nt + 1) * BC])

            # bias: E @ bp per node chunk (PE, tiny; bf16 moving side)
            with tc.tile_pool(name="psb", bufs=2, space="PSUM") as psb:
                for nt in range(NT):
                    pb = psb.tile([128, CO], F32, tag="pb")
                    nc.tensor.matmul(pb[:], etb[0:D, nt * 128:(nt + 1) * 128],
                                     bp_sb[0:D, :], start=True, stop=True)
                    nc.scalar.copy(bias_sb[:, nt * CO:(nt + 1) * CO], pb[:])

            # diagonal helper: Mexp_nn = exp(|E_n|^2)  (always >= 1)
            nc.vector.tensor_tensor(e2[:], e_sb[:], e_sb[:], op=ALU.mult)
            nc.vector.tensor_reduce(
                nrm[:].rearrange("p (t u) -> p t u", u=1),
                e2[:].rearrange("p (t d) -> p t d", t=NT),
                op=ALU.add, axis=mybir.AxisListType.X)
            nc.scalar.activation(diagm[:], nrm[:], AF.Exp)

            nc.vector.memset(ones[:], 1.0 / KA)

            # ---- phase 1: Mexp rows (bf16) + rowsums + fp8 A^T cast --------
            with (
                tc.tile_pool(name="mb", bufs=1) as mbpool,
                tc.tile_pool(name="mexps", bufs=3) as mxpool,
            ):
                mb = [mbpool.tile([128, N], BF16, name=f"mb{t}", tag=f"mb{t}")
                      for t in range(NT)]
                with (
                    tc.tile_pool(name="ps1", bufs=2, space="PSUM") as ps1,
                    tc.tile_pool(name="pcs", bufs=1, space="PSUM") as pcs,
                ):
                    # column sums of Mexp/128 (= row sums by symmetry),
                    # replicated to all 128 partitions by the ones lhsT
                    cs = pcs.tile([128, N], F32, name="cs")
                    for t in range(NT):
                        mx = mxpool.tile([128, N], BF16, tag="mx")
                        for half in range(2):
                            ps = ps1.tile([128, 1024], F32)
                            for j in range(2):
                                c0 = half * 1024 + j * 512
                                nc.tensor.matmul(
                                    ps[:, j * 512:(j + 1) * 512],
                                    et[0:D, t * 128:(t + 1) * 128],
                                    et[0:D, c0:c0 + 512],
                                    start=True, stop=True)
                            nc.scalar.activation(
                                mx[:, half * 1024:(half + 1) * 1024],
                                ps[:], AF.Exp)
                        # max(exp(s),1) = exp(relu(s))
                        nc.vector.tensor_scalar(mb[t][:], mx[:], 1.0, None,
                                                op0=ALU.max)
                        for c in range(4):
                            nc.tensor.matmul(
                                cs[:, c * 512:(c + 1) * 512], ones[:],
                                mb[t][:, c * 512:(c + 1) * 512],
                                start=(t == 0), stop=(t == NT - 1))

                    # rrep = 128/R[n], already on all partitions
                    with nc.allow_low_precision(reason="bf16 1/R scale"):
                        nc.vector.reciprocal(rrep[:], cs[:])

                # per-partition r: rd_s[p, w, j] = r[w*128+p] (col j=0 valid)
                nc.sync.dma_start_transpose(rdv, rrep[:])
                nc.vector.tensor_scalar(rd2[:], rdv[:, :, 0], 2.0 / KA, None,
                                        op0=ALU.mult)
                with nc.allow_low_precision(reason="bf16 diag scale"):
                    nc.vector.tensor_tensor(amn[:], diagm[:], rdv[:, :, 0],
                                            op=ALU.mult)
                nc.vector.tensor_scalar(amn[:], amn[:], 1.0 / KA, None,
                                        op0=ALU.mult)

                # cast: AT[:, mt*N+n] = Mb_mt[:, n] * rrep[n] (fp8), then
                # zero the diagonal block via (1-I) mask overwrite
                dgp = mxpool  # reuse pool for small diag scratch
                for quart in range(2):
                    cl, ch = quart * 1024, (quart + 1) * 1024
                    for mt in range(NT):
                        eng = nc.vector if mt % 8 < 5 else nc.gpsimd
                        eng.tensor_tensor(atv[:, mt, cl:ch],
                                          mb[mt][:, cl:ch], rrep[:, cl:ch],
                                          op=ALU.mult)
                        if cl <= mt * 128 < ch:
                            dg = dgp.tile([128, 128], BF16, tag="dg")
                            nc.gpsimd.tensor_tensor(
                                dg[:], mb[mt][:, mt * 128:(mt + 1) * 128],
                                rrep[:, mt * 128:(mt + 1) * 128],
                                op=ALU.mult)
                            nc.gpsimd.tensor_tensor(
                                atv[:, mt, mt * 128:(mt + 1) * 128], dg[:],
                                maskI[:], op=ALU.mult)

                # ---- phase 3: z1 = A x  (fp8 DoubleRow, x + x_residual) ----
                with (
                    tc.tile_pool(name="ps3", bufs=3, space="PSUM") as ps3,
                    tc.tile_pool(name="ztmp", bufs=3) as ztpool,
                ):
                    for nt in range(NT):
                        ps = ps3.tile([128, BC], F32)
                        for q in range(NT // 2):
                            nc.tensor.matmul(
                                ps[:],
                                atv[:, 2 * q:2 * q + 2,
                                    nt * 128:(nt + 1) * 128],
                                xfv[:, 2 * q:2 * q + 2, :],
                                start=(q == 0), stop=False, perf_mode=DR)
                        for q in range(NT // 2):
                            nc.tensor.matmul(
                                ps[:],
                                atv[:, 2 * q:2 * q + 2,
                                    nt * 128:(nt + 1) * 128],
                                xrv[:, 2 * q:2 * q + 2, :],
                                start=False, stop=(q == NT // 2 - 1),
                                perf_mode=DR)
                        # z1 = psum/128 + a*x  (exact diagonal path)
                        tmpx = ztpool.tile([128, BC], BF16, tag="tmpx")
                        nc.gpsimd.tensor_scalar(
                            tmpx[:].rearrange("p (b c) -> p b c", b=BL),
                            xbf[:, nt * BC:(nt + 1) * BC]
                            .rearrange("p (b c) -> p b c", b=BL),
                            amn[:, nt:nt + 1], None, op0=ALU.mult)
                        nc.vector.scalar_tensor_tensor(
                            z1bf[:, nt * BC:(nt + 1) * BC], ps[:], 1.0 / KA,
                            tmpx[:], op0=ALU.mult, op1=ALU.add)

                # ---- phase 4: z2 = 2 r (Mexp z1) - x ; z12t via xbar -------
                with (
                    tc.tile_pool(name="ps4", bufs=3, space="PSUM") as ps4,
                    tc.tile_pool(name="zc", bufs=3) as zcpool,
                ):
                    for nt in range(NT):
                        ps = ps4.tile([128, BC], F32)
                        for mt in range(NT):
                            nc.tensor.matmul(
                                ps[:], mb[mt][:, nt * 128:(nt + 1) * 128],
                                z1bf[:, mt * BC:(mt + 1) * BC],
                                start=(mt == 0), stop=(mt == NT - 1))
                        zc = zcpool.tile([128, BL * 128], BF16, tag="zc")
                        zcv = zc[:].rearrange("p (b k) -> p b k", b=BL)
                        nc.vector.tensor_copy(
                            zcv[:, :, 0:64],
                            z1bf[:, nt * BC:(nt + 1) * BC]
                            .rearrange("p (b c) -> p b c", b=BL))
                        nc.vector.scalar_tensor_tensor(
                            zcv[:, :, 64:128],
                            ps[:].rearrange("p (b c) -> p b c", b=BL),
                            rd2[:, nt:nt + 1],
                            xbf[:, nt * BC:(nt + 1) * BC]
                            .rearrange("p (b c) -> p b c", b=BL),
                            op0=ALU.mult, op1=ALU.subtract)
                        nc.sync.dma_start_transpose(
                            z12v[:, :, nt * 128:(nt + 1) * 128], zc[:])

            # ---- phase 5: y = zT.W ; out = sum_d E_d * y_d + bias ----------
            with (
                tc.tile_pool(name="psy", bufs=2, space="PSUM") as psy,
                tc.tile_pool(name="accp", bufs=3) as accpool,
            ):
                nev = 0
                for nt in range(NT):
                    acc = accpool.tile([128, BC], F32, tag="acc")
                    accv = acc[:].rearrange("p (q o) -> p q o", q=BL)
                    bsl = bias_sb[:, nt * CO:(nt + 1) * CO]
                    bsrc = bass.AP(bsl.tensor, bsl.offset,
                                   [list(bsl.ap[0]), [0, BL], [1, CO]])
                    ytmp = accpool.tile([128, BL * DO], BF16, tag="ytmp")
                    ytv = ytmp[:].rearrange("p (q f) -> p q f", q=BL)
                    for bp2 in range(BL // 2):          # b pairs
                        ps = psy.tile([128, 2048], F32)
                        for bh in range(2):
                            b = bp2 * 2 + bh
                            po = bh * 1024
                            hp = (b % 2) * 64
                            w = b // 2
                            for w0, f0 in ((0, 0), (512, 320)):
                                nc.tensor.matmul(
                                    ps[:, po + w0:po + w0 + 320],
                                    z12v[:, b, nt * 128:(nt + 1) * 128],
                                    wr12[:, f0:f0 + 320],
                                    start=True, stop=False)
                                nc.tensor.matmul(
                                    ps[:, po + w0:po + w0 + 320],
                                    z0v[hp:hp + 64, w,
                                        nt * 128:(nt + 1) * 128],
                                    wr0[hp:hp + 64, f0:f0 + 320],
                                    start=False, stop=True)
                        psv = (ps[:]
                               .rearrange("p (b h f) -> p b h f", b=2, h=2)
                               [:, :, :, 0:320])
                        yv = (ytv[:, bp2 * 2:bp2 * 2 + 2, :]
                              .rearrange("p q (h f) -> p q h f", h=2))
                        nc.scalar.copy(yv, psv)
                        nev += 1
                    # d-contraction: out = sum_d E[:,d]*y_d + bias.
                    ysc = accpool.tile([128, 7 * BC], BF16, tag="ysc")
                    for i, d in enumerate(DCON_POOL):
                        nc.gpsimd.tensor_scalar(
                            ysc[:, i * BC:(i + 1) * BC].rearrange(
                                "p (q o) -> p q o", q=BL),
                            ytv[:, :, d * CO:(d + 1) * CO],
                            e_sb[:, nt * D + d:nt * D + d + 1], None,
                            op0=ALU.mult)
                    # chain A: d0..d2 into acc (seeded by bias)
                    accB = accpool.tile([128, BC], F32, tag="accB")
                    accBv = accB[:].rearrange("p (q o) -> p q o", q=BL)
                    for d in DCON_DVE[:3]:
                        nc.vector.scalar_tensor_tensor(
                            accv, ytv[:, :, d * CO:(d + 1) * CO],
                            e_sb[:, nt * D + d:nt * D + d + 1],
                            bsrc if d == DCON_DVE[0] else accv,
                            op0=ALU.mult, op1=ALU.add)
                    # fold pool results: 5 -> 2 -> 1
                    nc.vector.tensor_tensor(
                        ysc[:, 5 * BC:6 * BC], ysc[:, 0:BC], ysc[:, BC:2 * BC],
                        op=ALU.add)
                    nc.vector.tensor_tensor(
                        ysc[:, 6 * BC:7 * BC], ysc[:, 2 * BC:3 * BC],
                        ysc[:, 3 * BC:4 * BC], op=ALU.add)
                    nc.vector.tensor_tensor(
                        ysc[:, 0:BC], ysc[:, 5 * BC:6 * BC],
                        ysc[:, 6 * BC:7 * BC], op=ALU.add)
                    nc.vector.tensor_tensor(
                        ysc[:, BC:2 * BC], ysc[:, 0:BC],
                        ysc[:, 4 * BC:5 * BC], op=ALU.add)
                    # chain B: d3, d4 on top of the pool fold
                    nc.vector.scalar_tensor_tensor(
                        accBv, ytv[:, :, DCON_DVE[3] * CO:(DCON_DVE[3] + 1) * CO],
                        e_sb[:, nt * D + DCON_DVE[3]:nt * D + DCON_DVE[3] + 1],
                        ysc[:, BC:2 * BC].rearrange("p (q o) -> p q o", q=BL),
                        op0=ALU.mult, op1=ALU.add)
                    nc.vector.scalar_tensor_tensor(
                        accBv, ytv[:, :, DCON_DVE[4] * CO:(DCON_DVE[4] + 1) * CO],
                        e_sb[:, nt * D + DCON_DVE[4]:nt * D + DCON_DVE[4] + 1],
                        accBv, op0=ALU.mult, op1=ALU.add)
                    nc.vector.tensor_tensor(accv, accv, accBv, op=ALU.add)
                    nc.gpsimd.dma_start(
                        out_d[nt * 128:(nt + 1) * 128, :, :], accv)

    split_multi_waits(nc)
    return nc


_NC_CACHE = None


def get_nc():
    global _NC_CACHE
    if _NC_CACHE is None:
        _NC_CACHE = build_nc()
    return _NC_CACHE


def make_in_maps(inputs):
    x = np.asarray(inputs["x"], dtype=np.float32)
    emb = np.ascontiguousarray(np.asarray(inputs["node_embeddings"],
                                          dtype=np.float32))
    wpa = np.ascontiguousarray(np.asarray(inputs["weights_pool"],
                                          dtype=np.float32))
    bpa = np.ascontiguousarray(np.asarray(inputs["bias_pool"],
                                          dtype=np.float32))
    embT = np.ascontiguousarray(emb.T)
    maskI = (1.0 - np.eye(128, dtype=np.float32)).astype(ml_dtypes.bfloat16)
    bpb = bpa.astype(ml_dtypes.bfloat16)
    maps = []
    for c in range(NCORES):
        xc = np.ascontiguousarray(
            x[c * BL:(c + 1) * BL].transpose(1, 0, 2).reshape(N, BC))
        xq = xc.astype(ml_dtypes.float8_e4m3)
        xr = (xc - xq.astype(np.float32)).astype(ml_dtypes.float8_e4m3)
        maps.append(dict(
            xbf=xc.astype(ml_dtypes.bfloat16),
            xf8=xq, xr8=xr,
            embT=embT, emb=emb, wp=wpa, bp=bpb, maskd=maskI))
    return maps


def kernel(**inputs) -> np.ndarray:
    nc = get_nc()
    res = run_bass_kernel_spmd(nc, make_in_maps(inputs), list(range(NCORES)))
    out = np.concatenate(
        [res.results[c]["out_l"].transpose(1, 0, 2) for c in range(NCORES)],
        axis=0)
    return out.astype(np.float32)


# revision 3
# speedup vs baseline: 1.0167x; 1.0167x over previous
"""AVWGCN Trainium2 kernel, v3: SBUF-resident graph conv, fp8 phase-3.

Math (per batch b, node n):
    S = E @ E.T                        [N, N]  (symmetric)
    Mexp = exp(relu(S)) = max(exp(S),1)
    R = rowsum(Mexp);  A = diag(1/R) Mexp     (softmax(relu(S)))
    z0 = x ; z1 = A x ; z2 = 2 A z1 - x
    out[b,n,o] = sum_d E[n,d] * (sum_ki z_k[b,n,i] Wp[d,k,i,o]) + (E bp)[n,o]

Design vs baseline:
  * Everything stays in SBUF (no Mexp DRAM spill).
  * Phase 3 (z1 = A x): A^T stored column-normalized fp8e4m3
    (AT[m,n] = 128*A[n,m], diagonal zeroed), DoubleRow fp8 matmuls over
    (x_fp8 + x_residual_fp8); exact diagonal a*x added on DVE.
  * Phase 4 (A z1) in bf16 against the raw symmetric Mexp tiles, with
    1/R applied per-partition in the stitch (z2 = 2*r*psum - x).
  * exp(relu(s)) computed as max(exp(s),1); rowsums via 1-partition
    ones-matmuls on PE (columns sums = row sums by symmetry).
  * All transposes via DMA xbar; host pre-transposes x / casts dtypes;
    output written [N, B_local, CO], untransposed on host.
"""
import sys

if "/opt/trn_rl_repo" not in sys.path:
    sys.path.insert(0, "/opt/trn_rl_repo")

import numpy as np
import ml_dtypes
import bass_rust
import concourse.bass as bass
import concourse.mybir as mybir
from concourse import tile
from concourse.vector_clock import ScopedClock
from concourse.bass_utils import run_bass_kernel_spmd

F32 = mybir.dt.float32
F32R = mybir.dt.float32r
BF16 = mybir.dt.bfloat16
F8E4 = mybir.dt.float8e4
ALU = mybir.AluOpType
AF = mybir.ActivationFunctionType
DR = mybir.MatmulPerfMode.DoubleRow

B, N, CI, CO, KCH, D = 64, 2048, 64, 64, 3, 10
NCORES = 8
BL = B // NCORES          # 8 local batches
BC = BL * CI              # 512
NT = N // 128             # 16 node chunks
DO = D * CO               # 640
KA = 128.0                # fp8 A scale

# engine splits (tunables)
EVAC_ACT = 8              # ACT evacs per 8 (all ACT)
DCON_DVE = (0, 1, 2, 3, 4)     # d's via DVE fused stt
DCON_POOL = (5, 6, 7, 8, 9)    # d's via Pool scale + DVE fold


# ---------------------------------------------------------------- env patches
def _patched_drain_and_barrier(self, tick_clock, wait_clock):
    nop_inst = self.nc.sync.nop(nofuse=True, hint="tile_tail_wait")
    wait_clock.add_sem_waits(nop_inst.ins,
                             ScopedClock({None: tick_clock.global_clock}))
    si = nop_inst.ins.sync_info
    waits = list(si.on_wait) if si is not None else []
    if len(waits) > 1:
        nop_inst.ins.sync_info = bass_rust.SyncInfo(
            on_wait=waits[:1], on_update=list(si.on_update))
        for w in waits[1:]:
            extra = self.nc.sync.nop(nofuse=True, hint="tile_tail_wait_x")
            extra.ins.sync_info = bass_rust.SyncInfo(on_wait=[w], on_update=[])
    self.nc.sync.drain()
    self.nc.all_engine_barrier()
    assert self.sems is not None
    popped = self.nc._tile_sem_poison_stack.pop()
    assert popped is self._sem_poison
    self.nc.clear_and_free_semaphores(list(self.sems.allocated().values()))
    self.nc.all_engine_barrier()


tile.TileContext._drain_and_barrier = _patched_drain_and_barrier


def split_multi_waits(nc):
    for f in nc.m.functions:
        for bb in f.blocks:
            new = []
            for inst in bb.instructions:
                si = inst.sync_info
                if si is not None and len(si.on_wait) > 1:
                    waits = list(si.on_wait)
                    for w in waits[:-1]:
                        nop = mybir.InstNoOp(
                            name=nc.get_next_instruction_name(), ins=[], outs=[])
                        nop.engine = inst.engine
                        nop.sync_info = bass_rust.SyncInfo(on_wait=[w], on_update=[])
                        new.append(nop)
                    inst.sync_info = bass_rust.SyncInfo(
                        on_wait=[waits[-1]], on_update=list(si.on_update))
                new.append(inst)
            bb.instructions = new


# ---------------------------------------------------------------- kernel body
def build_nc():
    nc = bass.Bass()

    xbf_d = nc.dram_tensor("xbf", [N, BC], BF16, kind="ExternalInput")
    xf8_d = nc.dram_tensor("xf8", [N, BC], F8E4, kind="ExternalInput")
    xr8_d = nc.dram_tensor("xr8", [N, BC], F8E4, kind="ExternalInput")
    embT_d = nc.dram_tensor("embT", [D, N], F32, kind="ExternalInput")
    emb_d = nc.dram_tensor("emb", [N, D], F32, kind="ExternalInput")
    wp_d = nc.dram_tensor("wp", [D, KCH, CI, CO], F32, kind="ExternalInput")
    bp_d = nc.dram_tensor("bp", [D, CO], BF16, kind="ExternalInput")
    mask_d = nc.dram_tensor("maskd", [128, 128], BF16, kind="ExternalInput")
    out_d = nc.dram_tensor("out_l", [N, BL, CO], F32, kind="ExternalOutput")

    with tile.TileContext(nc) as tc:
        with (
            tc.tile_pool(name="const", bufs=1) as cpool,
            tc.tile_pool(name="big", bufs=1) as bpool,
        ):
            # ---- constants -------------------------------------------------
            et = cpool.tile([16, N], F32R)             # E^T rows 0:10 (f32r)
            etb = cpool.tile([16, N], BF16)            # E^T bf16 (bias mm)
            e_sb = cpool.tile([128, NT * D], F32)      # E as [p, nt*10+d]
            bias_sb = cpool.tile([128, NT * CO], F32)
            bp_sb = cpool.tile([16, CO], BF16)
            wr12 = cpool.tile([128, DO], BF16)         # [W1;W2] stacked (ki)
            wr0 = cpool.tile([128, DO], BF16)          # W0 duplicated
            rrep = cpool.tile([128, N], BF16)          # 128/R[n], bcast rows
            rd_s = cpool.tile([128, N], BF16)          # xbar of rrep
            ones = cpool.tile([128, 128], BF16)        # 1/128 each
            maskI = cpool.tile([128, 128], BF16)       # 1 - I
            amn = cpool.tile([128, NT], F32)           # a[n] = Mexp_nn / R
            rd2 = cpool.tile([128, NT], F32)           # 2/R[n] ([p,t] layout)
            diagm = cpool.tile([128, NT], BF16)
            e2 = cpool.tile([128, NT * D], F32)
            nrm = cpool.tile([128, NT], F32)

            # ---- big persistent tiles -------------------------------------
            atb = bpool.tile([128, NT * N], F8E4, name="atb")   # A^T * 128
            xbf = bpool.tile([128, NT * BC], BF16, name="xbf")
            xf8 = bpool.tile([128, NT * BC], F8E4, name="xf8")
            xr8 = bpool.tile([128, NT * BC], F8E4, name="xr8")
            z1bf = bpool.tile([128, NT * BC], BF16, name="z1bf")
            z0t = bpool.tile([128, 4 * N], BF16, name="z0t")

            atv = atb[:].rearrange("p (mt n) -> p mt n", mt=NT)
            xfv = xf8[:].rearrange("p (mt c) -> p mt c", mt=NT)
            xrv = xr8[:].rearrange("p (mt c) -> p mt c", mt=NT)
            z0v = z0t[:].rearrange("p (w n) -> p w n", w=4)
            # z12t reuses atb's bytes after phase 3 is done with it
            z12t = atb[:].bitcast(BF16)
            z12v = z12t.rearrange("p (b n) -> p b n", b=BL)
            rdv = rd_s[:].rearrange("p (w j) -> p w j", w=NT)

            nc.sync.dma_start(et[0:D, :], embT_d[:].bitcast(F32R))
            nc.vector.tensor_copy(etb[0:D, :], et[0:D, :].bitcast(F32))
            nc.sync.dma_start(
                e_sb[:].rearrange("p (nt d) -> p nt d", nt=NT),
                emb_d[:].rearrange("(nt p) d -> p nt d", p=128))
            nc.sync.dma_start(bp_sb[0:D, :], bp_d[:])
            nc.sync.dma_start(maskI[:], mask_d[:])

            # x loads (SWDGE queue; SP stays free for phase-1 DMAs)
            for t, src in ((xbf, xbf_d), (xf8, xf8_d), (xr8, xr8_d)):
                nc.gpsimd.dma_start(
                    t[:].rearrange("p (mt c) -> p mt c", mt=NT),
                    src[:].rearrange("(mt p) c -> p mt c", p=128))

            # weights: load W0,W1,W2 as [i, (d,o)] then cast to bf16
            with tc.tile_pool(name="wload", bufs=1) as wpool:
                wtmp = []
                for k in range(KCH):
                    wf = wpool.tile([64, DO], F32, tag=f"wload{k}")
                    nc.sync.dma_start(
                        wf[:].rearrange("p (d o) -> p d o", d=D),
                        wp_d[:, k, :, :].transpose([1, 0, 2]))
                    wtmp.append(wf)
                nc.vector.tensor_copy(wr12[0:64, :], wtmp[1][:])
                nc.vector.tensor_copy(wr12[64:128, :], wtmp[2][:])
                nc.vector.tensor_copy(wr0[0:64, :], wtmp[0][:])
                nc.vector.tensor_copy(wr0[64:128, :], wtmp[0][:])

            # xT transposes into z0t (DMA xbar), as soon as xbf lands
            for nt in range(NT):
                nc.sync.dma_start_transpose(
                    z0v[:, :, nt * 128:(nt + 1) * 128],
                    xbf[:, nt * BC:(nt + 1) * BC])

            # bias: E @ bp per node chunk (PE, tiny; bf16 moving side)
            with tc.tile_pool(name="psb", bufs=2, space="PSUM") as psb:
                for nt in range(NT):
                    pb = psb.tile([128, CO], F32, tag="pb")
                    nc.tensor.matmul(pb[:], etb[0:D, nt * 128:(nt + 1) * 128],
                                     bp_sb[0:D, :], start=True, stop=True)
                    nc.scalar.copy(bias_sb[:, nt * CO:(nt + 1) * CO], pb[:])

            # diagonal helper: Mexp_nn = exp(|E_n|^2)  (always >= 1)
            nc.vector.tensor_tensor(e2[:], e_sb[:], e_sb[:], op=ALU.mult)
            nc.vector.tensor_reduce(
                nrm[:].rearrange("p (t u) -> p t u", u=1),
                e2[:].rearrange("p (t d) -> p t d", t=NT),
                op=ALU.add, axis=mybir.AxisListType.X)
            nc.scalar.activation(diagm[:], nrm[:], AF.Exp)

            nc.vector.memset(ones[:], 1.0 / KA)

            # ---- phase 1: Mexp rows (bf16) + rowsums + fp8 A^T cast --------
            with (
                tc.tile_pool(name="mb", bufs=1) as mbpool,
                tc.tile_pool(name="mexps", bufs=2) as mxpool,
            ):
                mb = [mbpool.tile([128, N], BF16, name=f"mb{t}", tag=f"mb{t}")
                      for t in range(NT)]
                with (
                    tc.tile_pool(name="ps1", bufs=2, space="PSUM") as ps1,
                    tc.tile_pool(name="pcs", bufs=1, space="PSUM") as pcs,
                ):
                    # column sums of Mexp/128 (= row sums by symmetry),
                    # replicated to all 128 partitions by the ones lhsT
                    cs = pcs.tile([128, N], F32, name="cs")
                    for t in range(NT):
                        mx = mxpool.tile([128, N], BF16, tag="mx")
                        for half in range(2):
                            ps = ps1.tile([128, 1024], F32)
                            for j in range(2):
                                c0 = half * 1024 + j * 512
                                nc.tensor.matmul(
                                    ps[:, j * 512:(j + 1) * 512],
                                    et[0:D, t * 128:(t + 1) * 128],
                                    et[0:D, c0:c0 + 512],
                                    start=True, stop=True)
                            nc.scalar.activation(
                                mx[:, half * 1024:(half + 1) * 1024],
                                ps[:], AF.Exp)
                        # max(exp(s),1) = exp(relu(s))
                        nc.vector.tensor_scalar(mb[t][:], mx[:], 1.0, None,
                                                op0=ALU.max)
                        for c in range(4):
                            nc.tensor.matmul(
                                cs[:, c * 512:(c + 1) * 512], ones[:],
                                mb[t][:, c * 512:(c + 1) * 512],
                                start=(t == 0), stop=(t == NT - 1))

                    # rrep = 128/R[n], already on all partitions
                    with nc.allow_low_precision(reason="bf16 1/R scale"):
                        nc.vector.reciprocal(rrep[:], cs[:])

                # per-partition r: rd_s[p, w, j] = r[w*128+p] (col j=0 valid)
                nc.sync.dma_start_transpose(rdv, rrep[:])
                nc.vector.tensor_scalar(rd2[:], rdv[:, :, 0], 2.0 / KA, None,
                                        op0=ALU.mult)
                with nc.allow_low_precision(reason="bf16 diag scale"):
                    nc.vector.tensor_tensor(amn[:], diagm[:], rdv[:, :, 0],
                                            op=ALU.mult)
                nc.vector.tensor_scalar(amn[:], amn[:], 1.0 / KA, None,
                                        op0=ALU.mult)

                # cast: AT[:, mt*N+n] = Mb_mt[:, n] * rrep[n] (fp8), then
                # zero the diagonal block via (1-I) mask overwrite
                dgp = mxpool  # reuse pool for small diag scratch
                for quart in range(2):
                    cl, ch = quart * 1024, (quart + 1) * 1024
                    for mt in range(NT):
                        eng = nc.vector if mt % 8 < 5 else nc.gpsimd
                        eng.tensor_tensor(atv[:, mt, cl:ch],
                                          mb[mt][:, cl:ch], rrep[:, cl:ch],
                                          op=ALU.mult)
                        if cl <= mt * 128 < ch:
                            dg = dgp.tile([128, 128], BF16, tag="dg")
                            nc.gpsimd.tensor_tensor(
                                dg[:], mb[mt][:, mt * 128:(mt + 1) * 128],
                                rrep[:, mt * 128:(mt + 1) * 128],
                                op=ALU.mult)
                            nc.gpsimd.tensor_tensor(
                                atv[:, mt, mt * 128:(mt + 1) * 128], dg[:],
                                maskI[:], op=ALU.mult)

                # ---- phase 3: z1 = A x  (fp8 DoubleRow, x + x_residual) ----
                with (
                    tc.tile_pool(name="ps3", bufs=3, space="PSUM") as ps3,
                    tc.tile_pool(name="ztmp", bufs=3) as ztpool,
                    tc.tile_pool(name="ps4", bufs=3, space="PSUM") as ps4,
                    tc.tile_pool(name="zc", bufs=3) as zcpool,
                ):
                    for nt in range(NT):
                        ps = ps3.tile([128, BC], F32)
                        for q in range(NT // 2):
                            nc.tensor.matmul(
                                ps[:],
                                atv[:, 2 * q:2 * q + 2,
                                    nt * 128:(nt + 1) * 128],
                                xfv[:, 2 * q:2 * q + 2, :],
                                start=(q == 0), stop=False, perf_mode=DR)
                        for q in range(NT // 2):
                            nc.tensor.matmul(
                                ps[:],
                                atv[:, 2 * q:2 * q + 2,
                                    nt * 128:(nt + 1) * 128],
                                xrv[:, 2 * q:2 * q + 2, :],
                                start=False, stop=(q == NT // 2 - 1),
                                perf_mode=DR)
                        # z1 = psum/128 + a*x  (exact diagonal path)
                        tmpx = ztpool.tile([128, BC], BF16, tag="tmpx")
                        nc.gpsimd.tensor_scalar(
                            tmpx[:].rearrange("p (b c) -> p b c", b=BL),
                            xbf[:, nt * BC:(nt + 1) * BC]
                            .rearrange("p (b c) -> p b c", b=BL),
                            amn[:, nt:nt + 1], None, op0=ALU.mult)
                        nc.vector.scalar_tensor_tensor(
                            z1bf[:, nt * BC:(nt + 1) * BC], ps[:], 1.0 / KA,
                            tmpx[:], op0=ALU.mult, op1=ALU.add)

                    # ---- phase 4: z2 = 2 r (Mexp z1) - x ; xbar --------
                    for nt in range(NT):
                        ps = ps4.tile([128, BC], F32)
                        for mt in range(NT):
                            nc.tensor.matmul(
                                ps[:], mb[mt][:, nt * 128:(nt + 1) * 128],
                                z1bf[:, mt * BC:(mt + 1) * BC],
                                start=(mt == 0), stop=(mt == NT - 1))
                        zc = zcpool.tile([128, BL * 128], BF16, tag="zc")
                        zcv = zc[:].rearrange("p (b k) -> p b k", b=BL)
                        nc.vector.tensor_copy(
                            zcv[:, :, 0:64],
                            z1bf[:, nt * BC:(nt + 1) * BC]
                            .rearrange("p (b c) -> p b c", b=BL))
                        nc.vector.scalar_tensor_tensor(
                            zcv[:, :, 64:128],
                            ps[:].rearrange("p (b c) -> p b c", b=BL),
                            rd2[:, nt:nt + 1],
                            xbf[:, nt * BC:(nt + 1) * BC]
                            .rearrange("p (b c) -> p b c", b=BL),
                            op0=ALU.mult, op1=ALU.subtract)
                        nc.sync.dma_start_transpose(
                            z12v[:, :, nt * 128:(nt + 1) * 128], zc[:])

            # ---- phase 5: y = zT.W ; out = sum_d E_d * y_d + bias ----------
            with (
                tc.tile_pool(name="psy", bufs=2, space="PSUM") as psy,
                tc.tile_pool(name="accp", bufs=3) as accpool,
            ):
                nev = 0
                for nt in range(NT):
                    acc = accpool.tile([128, BC], F32, tag="acc")
                    accv = acc[:].rearrange("p (q o) -> p q o", q=BL)
                    bsl = bias_sb[:, nt * CO:(nt + 1) * CO]
                    bsrc = bass.AP(bsl.tensor, bsl.offset,
                                   [list(bsl.ap[0]), [0, BL], [1, CO]])
                    ytmp = accpool.tile([128, BL * DO], BF16, tag="ytmp", bufs=4)
                    ytv = ytmp[:].rearrange("p (q f) -> p q f", q=BL)
                    for bp2 in range(BL // 2):          # b pairs
                        ps = psy.tile([128, 2048], F32)
                        for bh in range(2):
                            b = bp2 * 2 + bh
                            po = bh * 1024
                            hp = (b % 2) * 64
                            w = b // 2
                            for w0, f0 in ((0, 0), (512, 320)):
                                nc.tensor.matmul(
                                    ps[:, po + w0:po + w0 + 320],
                                    z12v[:, b, nt * 128:(nt + 1) * 128],
                                    wr12[:, f0:f0 + 320],
                                    start=True, stop=False)
                                nc.tensor.matmul(
                                    ps[:, po + w0:po + w0 + 320],
                                    z0v[hp:hp + 64, w,
                                        nt * 128:(nt + 1) * 128],
                                    wr0[hp:hp + 64, f0:f0 + 320],
                                    start=False, stop=True)
                        psv = (ps[:]
                               .rearrange("p (b h f) -> p b h f", b=2, h=2)
                               [:, :, :, 0:320])
                        yv = (ytv[:, bp2 * 2:bp2 * 2 + 2, :]
                              .rearrange("p q (h f) -> p q h f", h=2))
                        nc.scalar.copy(yv, psv)
                        nev += 1
                    # d-contraction: out = sum_d E[:,d]*y_d + bias.
                    ysc = accpool.tile([128, 7 * BC], BF16, tag="ysc")
                    for i, d in enumerate(DCON_POOL):
                        nc.gpsimd.tensor_scalar(
                            ysc[:, i * BC:(i + 1) * BC].rearrange(
                                "p (q o) -> p q o", q=BL),
                            ytv[:, :, d * CO:(d + 1) * CO],
                            e_sb[:, nt * D + d:nt * D + d + 1], None,
                            op0=ALU.mult)
                    # chain A: d0..d2 into acc (seeded by bias)
                    accB = accpool.tile([128, BC], F32, tag="accB")
                    accBv = accB[:].rearrange("p (q o) -> p q o", q=BL)
                    for d in DCON_DVE[:3]:
                        nc.vector.scalar_tensor_tensor(
                            accv, ytv[:, :, d * CO:(d + 1) * CO],
                            e_sb[:, nt * D + d:nt * D + d + 1],
                            bsrc if d == DCON_DVE[0] else accv,
                            op0=ALU.mult, op1=ALU.add)
                    # fold pool results: 5 -> 2 -> 1
                    nc.vector.tensor_tensor(
                        ysc[:, 5 * BC:6 * BC], ysc[:, 0:BC], ysc[:, BC:2 * BC],
                        op=ALU.add)
                    nc.vector.tensor_tensor(
                        ysc[:, 6 * BC:7 * BC], ysc[:, 2 * BC:3 * BC],
                        ysc[:, 3 * BC:4 * BC], op=ALU.add)
                    nc.vector.tensor_tensor(
                        ysc[:, 0:BC], ysc[:, 5 * BC:6 * BC],
                        ysc[:, 6 * BC:7 * BC], op=ALU.add)
                    nc.vector.tensor_tensor(
                        ysc[:, BC:2 * BC], ysc[:, 0:BC],
                        ysc[:, 4 * BC:5 * BC], op=ALU.add)
                    # chain B: d3, d4 on top of the pool fold
                    nc.vector.scalar_tensor_tensor(
                        accBv, ytv[:, :, DCON_DVE[3] * CO:(DCON_DVE[3] + 1) * CO],
                        e_sb[:, nt * D + DCON_DVE[3]:nt * D + DCON_DVE[3] + 1],
                        ysc[:, BC:2 * BC].rearrange("p (q o) -> p q o", q=BL),
                        op0=ALU.mult, op1=ALU.add)
                    nc.vector.scalar_tensor_tensor(
                        accBv, ytv[:, :, DCON_DVE[4] * CO:(DCON_DVE[4] + 1) * CO],
                        e_sb[:, nt * D + DCON_DVE[4]:nt * D + DCON_DVE[4] + 1],
                        accBv, op0=ALU.mult, op1=ALU.add)
                    nc.vector.tensor_tensor(accv, accv, accBv, op=ALU.add)
                    nc.gpsimd.dma_start(
                        out_d[nt * 128:(nt + 1) * 128, :, :], accv)

    split_multi_waits(nc)
    return nc


_NC_CACHE = None


def get_nc():
    global _NC_CACHE
    if _NC_CACHE is None:
        _NC_CACHE = build_nc()
    return _NC_CACHE


def make_in_maps(inputs):
    x = np.asarray(inputs["x"], dtype=np.float32)
    emb = np.ascontiguousarray(np.asarray(inputs["node_embeddings"],
                                          dtype=np.float32))
    wpa = np.ascontiguousarray(np.asarray(inputs["weights_pool"],
                                          dtype=np.float32))
    bpa = np.ascontiguousarray(np.asarray(inputs["bias_pool"],
                                          dtype=np.float32))
    embT = np.ascontiguousarray(emb.T)
    maskI = (1.0 - np.eye(128, dtype=np.float32)).astype(ml_dtypes.bfloat16)
    bpb = bpa.astype(ml_dtypes.bfloat16)
    maps = []
    for c in range(NCORES):
        xc = np.ascontiguousarray(
            x[c * BL:(c + 1) * BL].transpose(1, 0, 2).reshape(N, BC))
        xq = xc.astype(ml_dtypes.float8_e4m3)
        xr = (xc - xq.astype(np.float32)).astype(ml_dtypes.float8_e4m3)
        maps.append(dict(
            xbf=xc.astype(ml_dtypes.bfloat16),
            xf8=xq, xr8=xr,
            embT=embT, emb=emb, wp=wpa, bp=bpb, maskd=maskI))
    return maps


def kernel(**inputs) -> np.ndarray:
    nc = get_nc()
    res = run_bass_kernel_spmd(nc, make_in_maps(inputs), list(range(NCORES)))
    out = np.concatenate(
        [res.results[c]["out_l"].transpose(1, 0, 2) for c in range(NCORES)],
        axis=0)
    return out.astype(np.float32)
